# revision 5
# baseline (speedup 1.0000x reference)
"""Causal attention kernel for trn2, sharded over 8 NeuronCores.

Problem (B=4, S=2048, E=2048, H=16, D=128), fp32:
    qkv = x @ w_qkv; q,k,v = split(qkv)
    q,k,v reshaped (B,S,E)->(B,H,S,D) as a RAW view (no transpose), i.e.
    per (b,h): Q_h = rows [h*128,(h+1)*128) of q[b] reinterpreted [S,D].
    o = softmax(QK^T/sqrt(D) + causal(+1/-10000)) @ V, inverse raw view,
    out = o @ w_out.

Because the raw view maps head h to a contiguous block of 128 sequence
rows, the whole computation splits into B*H = 64 independent tasks, each
touching only x[b, h*128:(h+1)*128, :] and producing
out[b, h*128:(h+1)*128, :].  Core c gets 8 tasks = rows
[c*1024,(c+1)*1024) of x.reshape(B*S, E).  No collectives.

v5 (~600us vs 659us v4): v4 was 96.5% PE-occupied, so v5 cuts PE work:
the softmax-denominator ones-matmul no longer streams every pt tile
through the PE.  Instead groups of 4 pt tiles are pre-summed on
DVE/GpSimd (parallel ladder, f32) and the PE streams only the group
sums in f32r -- exact same accumulation math, 1/4 the den columns.
Plus: HAM warmup matmuls during the initial DMA wait, first wq tile
DMA chunked so the first real matmul starts ~7us earlier, psB at 4
PSUM banks, and the attention-gating final q copies split across
DVE+ACT.  fp8/DoubleRow was explored and measured numerically
infeasible for every tensor except the denominator (gate 2e-2;
q/k/v/out fp8 each cost 2.5-6e-2).
"""

import numpy as np

B, S, E = 4, 2048, 2048
H, D, P = 16, 128, 128
NCORES = 8
NT = 8                      # tasks per core (128 rows each)
ROWS = NT * P               # 1024 rows per core
SCALE = float(1.0 / np.sqrt(D))
NEG = -1.0e9  # pre-scale additive mask; exp underflows to exactly 0.0

_NC_CACHE = {}


def build_nc(den_grp=True, warmup=32, chunk_wq=True, split_last=True,
             psb_bufs=2, iters=1):
    import concourse.bass as bass
    import concourse.mybir as mybir
    import concourse.tile as tile
    from concourse import bacc

    f32 = mybir.dt.float32
    f32r = mybir.dt.float32r
    bf16 = mybir.dt.bfloat16
    AF = mybir.ActivationFunctionType
    ALU = mybir.AluOpType

    nc = bacc.Bacc("TRN2", target_bir_lowering=False, debug=False,
                   num_devices=NCORES)
    # xt: host-pretransposed x^T, layout [p=kk, kc, ti, m]
    xt = nc.dram_tensor("xt", [P, 16 * NT * P], bf16, kind="ExternalInput")
    # wqkv host layout [p, cbp(24), ko(16), 256]
    wqkv = nc.dram_tensor("wqkv", [P, 24 * 16 * 256], bf16,
                          kind="ExternalInput")
    # wout host layout [p, nch(4), co(16), 512]
    wout = nc.dram_tensor("wout", [P, 4 * 16 * 512], bf16,
                          kind="ExternalInput")
    out = nc.dram_tensor("out", [ROWS, E], f32, kind="ExternalOutput")

    xt_v = xt.ap().rearrange("p (kc t m) -> p kc (t m)", kc=16, t=NT)
    wq_v = wqkv.ap().rearrange("p (cbp ko c) -> p cbp ko c", cbp=24, ko=16)
    wo_v = wout.ap().rearrange("p (nch co n) -> p nch co n", nch=4, co=16)

    with tile.TileContext(nc) as tc:
        with (
            tc.tile_pool(name="const", bufs=1) as cpool,
            tc.tile_pool(name="persist", bufs=1) as ppool,
            tc.tile_pool(name="ot", bufs=NT) as otpool,
            tc.tile_pool(name="vn", bufs=2) as vnpool,
            tc.tile_pool(name="psA", bufs=4, space="PSUM") as psA,
            tc.tile_pool(name="psB", bufs=psb_bufs, space="PSUM") as psB,
        ):
            # maskT[kk, qq] = 0 where qq >= kk else NEG (transposed
            # orientation: partition = k, free = q).
            maskT = cpool.tile([P, P], f32, tag="maskT")
            nc.gpsimd.memset(maskT[:], 0.0)
            nc.gpsimd.affine_select(
                out=maskT[:], in_=maskT[:],
                compare_op=ALU.is_ge, fill=NEG,
                base=0, channel_multiplier=-1, pattern=[[1, P]],
            )
            # all-ones stationary for partition-sum (softmax denominator)
            ones = cpool.tile([P, P], bf16, tag="ones")
            nc.gpsimd.memset(ones[:], 1.0)
            onesf = cpool.tile([P, P], f32, tag="onesf")
            nc.gpsimd.memset(onesf[:], 1.0)
            onesr = cpool.tile([P, P], f32r, tag="onesr")
            nc.vector.tensor_copy(onesr[:], onesf[:])

            if warmup:
                # HAM warmup: keep the PE busy during the initial DMA
                # wait so the clock gate opens before the first real
                # matmul (saves the 1.2GHz cold window).
                wps = psA.tile([P, 512], f32, tag="mm512")
                for _ in range(warmup):
                    nc.tensor.matmul(wps[:, 0:P], ones[:], ones[:],
                                     start=True, stop=True,
                                     skip_group_check=True)

            for _ in range(iters):
                # Q^T/K^T per task: interleaved [d, ti, (i j)]
                qt_all = ppool.tile([P, NT, S], bf16, tag="qt")
                kt_all = ppool.tile([P, NT, S], bf16, tag="kt")
                ots = []
                with (
                    tc.tile_pool(name="vtp", bufs=1) as vtp,
                    tc.tile_pool(name="attw", bufs=4) as awpool,
                    tc.tile_pool(name="attd", bufs=3) as adpool,
                    tc.tile_pool(name="gsum", bufs=3) as gpool,
                ):
                  # V^T always interleaved [d, ti, (i j)]
                  vt_all = vtp.tile([P, NT, S], bf16, tag="vt")
                  with tc.tile_pool(name="qkv", bufs=1) as qpool:
                    # x^T resident: at8[kk, kc, ti*128+m]; issued on the
                    # scalar queue so it runs parallel to sync's wq loads.
                    at8 = qpool.tile([P, 16, NT * P], bf16, tag="at8")
                    for kcg in range(8):
                        nc.scalar.dma_start(
                            at8[:, kcg * 2:(kcg + 1) * 2, :],
                            xt_v[:, kcg * 2:(kcg + 1) * 2, :])

                    dsts = {0: qt_all, 1: kt_all, 2: vt_all}

                    # ------------- QKV phase -------------
                    # col-block order: V (32..47), K (16..31), Q (0..15)
                    # so V finishes first and vnat transposes overlap K/Q
                    # matmuls.
                    cbp_order = [20, 21, 22, 23, 16, 17, 18, 19,
                                 8, 9, 10, 11, 12, 13, 14, 15,
                                 0, 1, 2, 3, 4, 5, 6, 7]
                    with tc.tile_pool(name="wqst", bufs=2) as wst:
                        for cbi, cbp in enumerate(cbp_order):
                            wq = wst.tile([P, 16, 256], bf16, tag="wq")
                            if chunk_wq and cbi == 0:
                                # first weight tile chunked so the first
                                # matmul waits on 256KB, not 1MB
                                for kq in range(4):
                                    nc.sync.dma_start(
                                        wq[:, 4 * kq:4 * (kq + 1), :],
                                        wq_v[:, cbp,
                                             4 * kq:4 * (kq + 1), :])
                            else:
                                nc.sync.dma_start(wq[:], wq_v[:, cbp, :, :])
                            for ci in range(2):
                                cb = cbp * 2 + ci
                                j = cb % 16
                                dst = dsts[cb // 16]
                                ps0 = psA.tile([P, 512], f32, tag="mm512")
                                ps1 = psA.tile([P, 512], f32, tag="mm512")
                                # kc-outer so consecutive matmuls share
                                # the stationary operand
                                for kc in range(16):
                                    for hf, ps in ((0, ps0), (1, ps1)):
                                        nc.tensor.matmul(
                                            ps[:],
                                            wq[:, kc, ci * P:(ci + 1) * P],
                                            at8[:, kc,
                                                hf * 512:(hf + 1) * 512],
                                            start=(kc == 0),
                                            stop=(kc == 15))
                                for hf, ps in ((0, ps0), (1, ps1)):
                                    dv = dst.rearrange(
                                        "d t (i j) -> d t i j", j=16)[
                                        :, hf * 4:(hf + 1) * 4, :, j]
                                    src = ps[:].rearrange(
                                        "d (t i) -> d t i", t=4)
                                    # alternate engines: the strided
                                    # interleave write is the QKV-phase
                                    # bottleneck, split it DVE/ACT
                                    if (split_last and cbi >= 22
                                            and hf == 0):
                                        # these copies gate the first
                                        # attention matmuls; halve the
                                        # latency by using both engines
                                        nc.vector.tensor_copy(
                                            dv[:, 0:2], src[:, 0:2])
                                        nc.scalar.copy(
                                            dv[:, 2:4], src[:, 2:4])
                                    elif (cb * 2 + hf) % 2 == 0:
                                        nc.vector.tensor_copy(dv, src)
                                    else:
                                        nc.scalar.copy(dv, src)

                  with tc.tile_pool(name="oproj", bufs=2) as opool:
                    # -------------- attention (per task) --------------
                    for ti in range(NT):
                        # V natural [k, kt, d] for this task
                        vnat = vnpool.tile([P, 16, P], bf16, tag="vnat",
                                           name=f"vn{ti}")
                        nc.sync.dma_start_transpose(
                            vnat[:], vt_all[:, ti, :])
                        ot = otpool.tile([P, 16, P], bf16, tag="ot",
                                         name=f"ot{ti}")
                        ots.append(ot)
                        qt_t = qt_all[:, ti, :]
                        kt_t = kt_all[:, ti, :]

                        for qc in range(4):
                            ot_ps = psB.tile([P, 512], f32, tag="otacc")
                            den_ps = psB.tile([P, 512], f32, tag="denacc")
                            nkt = qc * 4 + 4
                            ngrp = nkt // 4 if den_grp else 0
                            gsum = None
                            for kt in range(nkt):
                                r = kt - qc * 4  # >=0: diagonal tile row
                                w0 = P * r if r > 0 else 0
                                s_ps = psA.tile([P, 512], f32, tag="mm512")
                                nc.tensor.matmul(
                                    s_ps[:, w0:512],
                                    kt_t[:, kt * P:(kt + 1) * P],
                                    qt_t[:, qc * 512 + w0:(qc + 1) * 512],
                                    start=True, stop=True)
                                pt = awpool.tile([P, 512], bf16, tag="pt")
                                if r >= 0:
                                    rr = P * r
                                    nc.vector.tensor_tensor(
                                        s_ps[:, rr:rr + P],
                                        s_ps[:, rr:rr + P],
                                        maskT[:], ALU.add)
                                nc.scalar.activation(
                                    pt[:, w0:512], s_ps[:, w0:512],
                                    AF.Exp, bias=1.0, scale=SCALE)
                                nc.tensor.matmul(
                                    ot_ps[:, w0:512], vnat[:, kt, :],
                                    pt[:, w0:512],
                                    start=(kt == 0), stop=(kt == nkt - 1),
                                    skip_group_check=(w0 > 0))
                                if den_grp:
                                    # pre-sum groups of 4 pt tiles off
                                    # the PE (parallel ladder); the den
                                    # matmul streams only group sums.
                                    gi = kt % 4
                                    g = kt // 4
                                    eng = (nc.vector, nc.gpsimd)[g % 2]
                                    if gi == 0:
                                        gsum = gpool.tile(
                                            [P, 512], f32r, tag="gs")
                                        eng.tensor_copy(gsum[:], pt[:])
                                    else:
                                        eng.tensor_tensor(
                                            gsum[:, w0:512],
                                            gsum[:, w0:512],
                                            pt[:, w0:512], ALU.add)
                                    if gi == 3:
                                        nc.tensor.matmul(
                                            den_ps[:], onesr[:], gsum[:],
                                            start=(g == 0),
                                            stop=(g == ngrp - 1))
                                else:
                                    nc.tensor.matmul(
                                        den_ps[:, w0:512], ones[:],
                                        pt[:, w0:512],
                                        start=(kt == 0),
                                        stop=(kt == nkt - 1),
                                        skip_group_check=(w0 > 0))
                            rec = adpool.tile([P, 512], f32, tag="rec")
                            nc.vector.reciprocal_approx_fast(
                                rec[:], den_ps[:])
                            nc.vector.tensor_tensor(
                                ot[:, qc * 4:(qc + 1) * 4, :].rearrange(
                                    "p s d -> p (s d)"),
                                ot_ps[:], rec[:], ALU.mult)

                    # ---------------- output projection ----------------
                    for nch in range(4):
                        wo = opool.tile([P, 16, 512], bf16, tag="wo")
                        nc.sync.dma_start(wo[:], wo_v[:, nch, :, :])
                        for ti in range(NT):
                            lt = ots[ti].rearrange(
                                "d qt (i j) -> d qt i j", j=16)
                            ps = psA.tile([P, 512], f32, tag="mm512")
                            for cc in range(16):
                                nc.tensor.matmul(
                                    ps[:], lt[:, :, :, cc],
                                    wo[:, cc, :],
                                    start=(cc == 0), stop=(cc == 15))
                            osb = opool.tile([P, 512], f32, tag="osb")
                            nc.scalar.copy(osb[:], ps[:])
                            nc.scalar.dma_start(
                                out.ap()[ti * P:(ti + 1) * P,
                                         nch * 512:(nch + 1) * 512],
                                osb[:])
    nc.compile()
    return nc


def _env_opts():
    import os
    opts = {}
    for k in ("den_grp", "chunk_wq", "split_last"):
        v = os.environ.get("BK_" + k.upper())
        if v is not None:
            opts[k] = v not in ("0", "false", "False")
    for k in ("warmup", "psb_bufs"):
        v = os.environ.get("BK_" + k.upper())
        if v is not None:
            opts[k] = int(v)
    return opts


def get_nc(**kw):
    opts = _env_opts()
    opts.update(kw)
    key = tuple(sorted(opts.items()))
    if key not in _NC_CACHE:
        _NC_CACHE[key] = build_nc(**opts)
    return _NC_CACHE[key]


def _prep_inputs(x, w_qkv, w_out):
    """Host-side dtype conversion + layout pre-packing (bf16)."""
    import ml_dtypes

    bf = ml_dtypes.bfloat16
    x = np.asarray(x, dtype=np.float32).reshape(NCORES, NT, P, 16, P)
    # xt[core][p, kc, ti, m] = x[core, ti, m, kc, p]
    xt = np.ascontiguousarray(x.transpose(0, 4, 3, 1, 2)).astype(bf)
    xt = xt.reshape(NCORES, P, 16 * NT * P)
    w = np.asarray(w_qkv, dtype=np.float32).reshape(16, P, 24, 256)
    wq = np.ascontiguousarray(w.transpose(1, 2, 0, 3)).astype(bf)
    wq = wq.reshape(P, 24 * 16 * 256)
    wo = np.asarray(w_out, dtype=np.float32).reshape(16, P, 4, 512)
    wo = np.ascontiguousarray(wo.transpose(1, 2, 0, 3)).astype(bf)
    wo = wo.reshape(P, 4 * 16 * 512)
    return xt, wq, wo


def kernel(x, w_qkv, w_out):
    from concourse.bass_utils import run_bass_kernel_spmd

    xt, wq, wo = _prep_inputs(x, w_qkv, w_out)
    nc = get_nc()
    in_maps = [
        {"xt": np.ascontiguousarray(xt[c]), "wqkv": wq, "wout": wo}
        for c in range(NCORES)
    ]
    res = run_bass_kernel_spmd(nc, in_maps, core_ids=list(range(NCORES)))
    outs = [res.results[c]["out"] for c in range(NCORES)]
    return np.concatenate(outs, axis=0).reshape(B, S, E).astype(np.float32)
